# revision 7
# baseline (speedup 1.0000x reference)
"""Trainium2 Bass kernel for ContextQueryAttention (BiDAF-style trilinear
attention). Data-parallel over batch across 8 NeuronCores (4 batches/core).

Per batch (c=1024 context rows, q=128 query rows, h=256 hidden):
  S[c,q]   = ctx@cw + (qry@qw)^T + (ctx*cqw)@qry^T + bias
  S_bar    = softmax_c(S); S_bar_bar = softmax_q(S)
  A        = S @ qry
  B        = S_bar @ (S_bar_bar^T @ ctx)
  out      = concat([ctx, A, ctx*A, ctx*B], -1)

On-chip strategy: transpose ctx/qry with the PE so the h-contraction runs as
fp32r matmuls; compute S^T [q,c] with s0/s1/bias folded in via a K=2
augmented matmul; one exp pass (ACT, fused row-sums) serves both softmaxes;
softmax_q normalizers are folded into the T-matmul's rhs (scale ctx rows),
softmax_c normalizers into T itself, so the unnormalized exp matrix is the
lhsT for both the T and B matmuls.
"""

import numpy as np

B, C, Q, H = 32, 1024, 128, 256
N_CORES = 8
BPC = B // N_CORES  # batches per core
P = 128
HC = H // P  # h chunks of 128
CT = C // P  # c tiles of 128
CCH = 512  # S^T free-dim chunk (1 PSUM bank of fp32)
NCC = C // CCH

_NC_CACHE = {}


def _build_kernel():
    import concourse.bacc as bacc
    import concourse.tile as tile
    from concourse import mybir
    from concourse.masks import make_identity

    f32 = mybir.dt.float32
    f32r = mybir.dt.float32r
    AF = mybir.ActivationFunctionType
    AX = mybir.AxisListType

    nc = bacc.Bacc(trn_type="TRN2", target_bir_lowering=False, debug=False)
    ctx_d = nc.dram_tensor("ctx", [BPC, C, H], f32, kind="ExternalInput").ap()
    qry_d = nc.dram_tensor("qry", [BPC, Q, H], f32, kind="ExternalInput").ap()
    cw_d = nc.dram_tensor("cw", [H], f32, kind="ExternalInput").ap()
    qw_d = nc.dram_tensor("qw", [H], f32, kind="ExternalInput").ap()
    cqw_d = nc.dram_tensor("cqw", [H], f32, kind="ExternalInput").ap()
    bias_d = nc.dram_tensor("bias", [1, 1], f32, kind="ExternalInput").ap()
    out_d = nc.dram_tensor("out", [BPC, C, 4 * H], f32, kind="ExternalOutput").ap()

    from contextlib import ExitStack

    with tile.TileContext(nc) as tc, ExitStack() as es:
        consts = es.enter_context(tc.tile_pool(name="consts", bufs=1))
        p_ctx = es.enter_context(tc.tile_pool(name="p_ctx", bufs=2))
        p_ctxT = es.enter_context(tc.tile_pool(name="p_ctxT", bufs=2))
        p_q = es.enter_context(tc.tile_pool(name="p_q", bufs=2))
        p_big = es.enter_context(tc.tile_pool(name="p_big", bufs=2))
        p_med = es.enter_context(tc.tile_pool(name="p_med", bufs=2))
        p_aug = es.enter_context(tc.tile_pool(name="p_aug", bufs=2))
        p_out = es.enter_context(tc.tile_pool(name="p_out", bufs=2))
        pp_tr = es.enter_context(tc.tile_pool(name="pp_tr", bufs=2, space="PSUM"))
        pp_st = es.enter_context(tc.tile_pool(name="pp_st", bufs=2, space="PSUM"))
        pp_mm = es.enter_context(tc.tile_pool(name="pp_mm", bufs=2, space="PSUM"))
        pp_t = es.enter_context(tc.tile_pool(name="pp_t", bufs=2, space="PSUM"))

        identity = consts.tile([P, P], f32)
        make_identity(nc, identity)
        cw_col = consts.tile([P, HC], f32)
        nc.sync.dma_start(out=cw_col, in_=cw_d.rearrange("(j p) -> p j", p=P))
        qw_col = consts.tile([P, HC], f32)
        nc.sync.dma_start(out=qw_col, in_=qw_d.rearrange("(j p) -> p j", p=P))
        cq_col = consts.tile([P, HC], f32)
        nc.sync.dma_start(out=cq_col, in_=cqw_d.rearrange("(j p) -> p j", p=P))
        bias_sb = consts.tile([1, 1], f32)
        nc.sync.dma_start(out=bias_sb, in_=bias_d)
        ones_c_f = consts.tile([1, C], f32)
        nc.vector.memset(ones_c_f, 1.0)
        ones_q = consts.tile([1, Q], f32r)
        nc.vector.tensor_copy(ones_q, ones_c_f[:, 0:Q])
        ones_c = consts.tile([1, C], f32r)
        nc.vector.tensor_copy(ones_c, ones_c_f)

        cw_colr = consts.tile([P, HC], f32r)
        nc.vector.tensor_copy(cw_colr, cw_col)
        qw_colr = consts.tile([P, HC], f32r)
        nc.vector.tensor_copy(qw_colr, qw_col)

        # all 4 batches of query in one DMA: [q, b, h]
        q_all = consts.tile([P, BPC, H], f32)
        nc.sync.dma_start(out=q_all, in_=qry_d.rearrange("b q h -> q b h"))
        q_all_r = consts.tile([P, BPC, H], f32r)
        nc.vector.tensor_copy(q_all_r, q_all)

        for b in range(BPC):
            qry = q_all[:, b, :]

            ctx_nat = p_ctx.tile([P, CT, H], f32, tag="ctx_nat")
            nc.sync.dma_start(
                out=ctx_nat, in_=ctx_d[b].rearrange("(t p) h -> p t h", p=P)
            )

            # ---- transpose query; apply cq_weight to the transposed copy ----
            qt_raw = p_q.tile([P, HC, Q], f32r, tag="qt_raw")
            qt_cq = p_q.tile([P, HC, Q], f32r, tag="qt_cq")
            for j in range(HC):
                pt = pp_tr.tile([P, P], f32, tag="tr")
                nc.tensor.transpose(pt, qry[:, j * P : (j + 1) * P], identity)
                nc.scalar.copy(qt_raw[:, j], pt)
                nc.vector.tensor_scalar_mul(qt_cq[:, j], pt, cq_col[:, j : j + 1])

            # ---- transpose context: ctxT[h, c] ----
            ctxT = p_ctxT.tile([P, HC, C], f32r, tag="ctxT")
            for t in range(CT):
                for j in range(HC):
                    pt = pp_tr.tile([P, P], f32, tag="tr")
                    nc.tensor.transpose(pt, ctx_nat[:, t, j * P : (j + 1) * P], identity)
                    nc.scalar.copy(ctxT[:, j, t * P : (t + 1) * P], pt)

            # ---- s1 row = (qry @ qw)^T as [1, q] ----
            s1p = pp_st.tile([1, Q], f32, tag="stp")
            for j in range(HC):
                nc.tensor.matmul(
                    s1p,
                    lhsT=qw_colr[:, j : j + 1],
                    rhs=qt_raw[:, j],
                    start=(j == 0),
                    stop=(j == HC - 1),
                )
            s1_row = p_aug.tile([1, Q], f32r, tag="s1_row")
            nc.scalar.copy(s1_row, s1p)

            # ---- s0 row = ctx @ cw (+bias) as [1, c] ----
            s0_row = p_aug.tile([1, C], f32r, tag="s0_row")
            for cc in range(NCC):
                s0p = pp_st.tile([1, CCH], f32, tag="stp")
                for j in range(HC):
                    nc.tensor.matmul(
                        s0p,
                        lhsT=cw_colr[:, j : j + 1],
                        rhs=ctxT[:, j, cc * CCH : (cc + 1) * CCH],
                        start=(j == 0),
                        stop=(j == HC - 1),
                    )
                nc.scalar.activation(
                    s0_row[0:1, cc * CCH : (cc + 1) * CCH],
                    s0p,
                    AF.Identity,
                    bias=bias_sb[0:1, :],
                    scale=1.0,
                )

            # ---- S^T [q, c] = qt_cq.T @ ctxT + aug; exp + raw copy ----
            e_t = p_big.tile([P, C], f32r, tag="e_t")
            st_raw = p_big.tile([P, C], f32r, tag="st_raw")
            rsum = p_aug.tile([P, NCC], f32, tag="rsum")
            for cc in range(NCC):
                sl = slice(cc * CCH, (cc + 1) * CCH)
                stp = pp_st.tile([P, CCH], f32, tag="stp")
                for j in range(HC):
                    nc.tensor.matmul(
                        stp,
                        lhsT=qt_cq[:, j],
                        rhs=ctxT[:, j, sl],
                        start=(j == 0),
                        stop=False,
                    )
                nc.tensor.matmul(
                    stp,
                    lhsT=s1_row,
                    rhs=ones_c[:, sl],
                    start=False,
                    stop=False,
                )
                nc.tensor.matmul(
                    stp,
                    lhsT=ones_q,
                    rhs=s0_row[:, sl],
                    start=False,
                    stop=True,
                )
                nc.scalar.activation(
                    e_t[:, sl], stp, AF.Exp, accum_out=rsum[:, cc : cc + 1]
                )
                nc.vector.tensor_copy(st_raw[:, sl], stp)

            # softmax_c denominators: rq[q] = 1 / sum_c exp
            zq = p_aug.tile([P, 1], f32, tag="zq")
            nc.vector.reduce_sum(zq, rsum, axis=AX.X)
            rq = p_aug.tile([P, 1], f32, tag="rq")
            nc.vector.reciprocal(rq, zq)

            # ---- E-transpose per c-tile; softmax_q normalizers into ctx ----
            e_sb = p_med.tile([P, CT, P], f32r, tag="e_sb")
            ctx_s = p_med.tile([P, CT, H], f32r, tag="ctx_s")
            zc = p_aug.tile([P, CT], f32, tag="zc")
            rc = p_aug.tile([P, CT], f32, tag="rc")
            for t in range(CT):
                pe_ = pp_tr.tile([P, P], f32, tag="tr")
                nc.tensor.transpose(pe_, e_t[:, t * P : (t + 1) * P].bitcast(f32), identity)
                nc.vector.reduce_sum(zc[:, t : t + 1], pe_, axis=AX.X)
                nc.vector.reciprocal(rc[:, t : t + 1], zc[:, t : t + 1])
                nc.scalar.copy(e_sb[:, t, :], pe_)
                nc.vector.tensor_scalar_mul(
                    ctx_s[:, t, :], ctx_nat[:, t, :], rc[:, t : t + 1]
                )

            # ---- T = S_bar_bar^T @ ctx as one tight accumulation group ----
            t_acc = pp_t.tile([P, H], f32, tag="t_acc")
            for t in range(CT):
                nc.tensor.matmul(
                    t_acc,
                    lhsT=e_sb[:, t, :],
                    rhs=ctx_s[:, t, :],
                    start=(t == 0),
                    stop=(t == CT - 1),
                )
            # fold softmax_c normalizer into T
            ts = p_med.tile([P, H], f32r, tag="ts")
            nc.vector.tensor_scalar_mul(ts, t_acc, rq)

            # ---- A and B per c-tile; assemble output channels ----
            out_t = p_out.tile([P, CT, 3 * H], f32, tag="out_t")
            for t in range(CT):
                sl = slice(t * P, (t + 1) * P)
                pa = pp_mm.tile([P, H], f32, tag="ab")
                nc.tensor.matmul(
                    pa,
                    lhsT=st_raw[:, sl],
                    rhs=q_all_r[:, b, :],
                    start=True,
                    stop=True,
                )
                nc.scalar.copy(out_t[:, t, 0:H], pa)
                nc.vector.tensor_mul(out_t[:, t, H : 2 * H], ctx_nat[:, t, :], pa)
                pb = pp_mm.tile([P, H], f32, tag="ab")
                nc.tensor.matmul(
                    pb,
                    lhsT=e_t[:, sl],
                    rhs=ts,
                    start=True,
                    stop=True,
                )
                nc.vector.tensor_mul(out_t[:, t, 2 * H : 3 * H], ctx_nat[:, t, :], pb)

            # ---- stores ----
            nc.sync.dma_start(
                out=out_d[b, :, 0:H].rearrange("(t p) h -> p t h", p=P), in_=ctx_nat
            )
            nc.sync.dma_start(
                out=out_d[b, :, H : 4 * H].rearrange("(t p) h -> p t h", p=P),
                in_=out_t,
            )

    nc.compile()
    return nc


def _get_nc():
    if "nc" not in _NC_CACHE:
        _NC_CACHE["nc"] = _build_kernel()
    return _NC_CACHE["nc"]


def kernel(context, query, c_mask, q_mask, c_weight, q_weight, cq_weight, bias):
    from concourse import bass_utils

    nc = _get_nc()
    context = np.ascontiguousarray(np.asarray(context, dtype=np.float32))
    query = np.ascontiguousarray(np.asarray(query, dtype=np.float32))
    cw = np.asarray(c_weight, dtype=np.float32).reshape(H).copy()
    qw = np.asarray(q_weight, dtype=np.float32).reshape(H).copy()
    cqw = np.asarray(cq_weight, dtype=np.float32).reshape(H).copy()
    bs = np.asarray(bias, dtype=np.float32).reshape(1, 1).copy()

    in_maps = []
    for i in range(N_CORES):
        sl = slice(i * BPC, (i + 1) * BPC)
        in_maps.append(
            {
                "ctx": np.ascontiguousarray(context[sl]),
                "qry": np.ascontiguousarray(query[sl]),
                "cw": cw,
                "qw": qw,
                "cqw": cqw,
                "bias": bs,
            }
        )

    res = bass_utils.run_bass_kernel_spmd(nc, in_maps, core_ids=list(range(N_CORES)))
    return np.concatenate([res.results[i]["out"] for i in range(N_CORES)], axis=0)
